# revision 25
# baseline (speedup 1.0000x reference)
"""Trainium2 Bass kernel for nn_EqvSelfAttention (B=4, N=1024, D=256, H=8).

Sharding: data-parallel over (batch b, query-half) -> 8 cores.
Each core computes all 8 heads for its 512 query rows against all 1024 keys.

Math notes (vs reference):
  * 1/sqrt(D)=1/16 folded into Wq (exact power of two).
  * Per-head location-bias MLP: loc_h = sum_d wg2[h,d]*relu(hid_hd) + bg2[h].
    - |wg2| folded into layer-1 weights/bias; sign applied in the PE
      "reduce" matmul that accumulates loc directly onto the content
      logits in PSUM (transposed layout [key, query]).
    - bg2 dropped: constant across keys => softmax-invariant.
  * Keys are processed in 9 units of 126 (last unit zero-padded with
    pk=0 dummy keys) so the MLP hidden fits 3 slots/key * 42 keys = 126
    partitions with NO padding slot: 3 hidden chunks per unit instead of
    4 per 128 keys => ~20% less PE + relu work than 4-slot packing.
  * Softmax computed without max subtraction (logits are O(+-6)). Key
    presence mask folded into V'' = [pk*V | pk]; the 33rd column of the
    A@V'' matmul yields the softmax denominator Z. Dummy keys have
    finite logits and pk=0, so they drop out of both sums.
  * The A@V'' matmul uses exp(logits) chunks as the stationary operand so
    the output lands ROW-major ([query, dh]); Z then sits in a per-query
    column and the whole softmax-divide + presence blend is cheap
    per-partition column math (no cross-partition reciprocal/replicate).
  * Absent queries (pq=0) produce uniform attention over ALL keys in the
    reference => Oh = mean(V). Handled by the (1-pq)*mean(V) blend term.
  * fp16 everywhere on the hot path (PE runs fp16 at 4x the fp32 rate);
    logits/softmax accumulate in fp32 PSUM. Verified rel err ~4e-4.
  * PSUM `start=True` resets the whole bank, so multi-region banks (av)
    only carry it on the first matmul into the bank.
"""

import sys
import numpy as np

sys.path.insert(0, "/opt/trn_rl_repo")

B, N, D, H, DH = 4, 1024, 256, 8, 32
R = 512   # query rows per core
U = 9     # key units of 126 (last padded)
KU = 126  # keys per unit
NCORES = 8

_CACHE = {}


def _build_program():
    from contextlib import ExitStack

    from concourse import bass, mybir
    import concourse.tile as tile
    from concourse.masks import make_identity

    f32 = mybir.dt.float32
    f16 = mybir.dt.float16
    AF = mybir.ActivationFunctionType
    OP = mybir.AluOpType
    ds = bass.ds

    nc = bass.Bass("TRN2", target_bir_lowering=False, debug=False)

    # ---- I/O declarations ----
    d_yt = nc.declare_dram_parameter("yt", [D, N], f16, isOutput=False)
    d_yqt = nc.declare_dram_parameter("yqt", [D, R], f16, isOutput=False)
    d_xp = nc.declare_dram_parameter("xp", [R, 3 * N], f16, isOutput=False)
    d_pku = nc.declare_dram_parameter("pku", [128, U], f32, isOutput=False)
    d_mre = nc.declare_dram_parameter("mre", [128, U], f16, isOutput=False)
    d_pq2 = nc.declare_dram_parameter("pq2", [128, 4], f32, isOutput=False)
    d_pqc2 = nc.declare_dram_parameter("pqc2", [128, 4], f32, isOutput=False)
    d_wq = nc.declare_dram_parameter("wq", [D, D], f16, isOutput=False)
    d_wk = nc.declare_dram_parameter("wk", [D, D], f16, isOutput=False)
    d_wv = nc.declare_dram_parameter("wv", [D, D], f16, isOutput=False)
    d_wo = nc.declare_dram_parameter("wo", [D, D], f16, isOutput=False)
    d_b4 = nc.declare_dram_parameter("b4", [4, D], f16, isOutput=False)
    d_bd = nc.declare_dram_parameter("bd", [H, KU, KU], f16, isOutput=False)
    d_rb = nc.declare_dram_parameter("rb", [KU, H], f32, isOutput=False)
    d_lr = nc.declare_dram_parameter("lr", [KU, H, 3, 128], f16, isOutput=False)
    d_o = nc.declare_dram_parameter("o", [R, D], f32, isOutput=True)

    with tile.TileContext(nc) as tc:
        with ExitStack() as ctx:
            consts = ctx.enter_context(tc.tile_pool(name="consts", bufs=1))
            persist = ctx.enter_context(tc.tile_pool(name="persist", bufs=1))

            # ---------- constants ----------
            ident = consts.tile([128, 128], f16)
            make_identity(nc, ident)
            ones512 = consts.tile([1, 512], f16)
            nc.vector.memset(ones512, 1.0)
            ones128 = consts.tile([1, 128], f16)
            nc.vector.memset(ones128, 1.0)

            wqs = consts.tile([128, 2, D], f16)
            wks = consts.tile([128, 2, D], f16)
            wvs = consts.tile([128, 2, D], f16)
            wos = consts.tile([128, 2, D], f16)
            b4s = consts.tile([1, 4, D], f16)  # bq, bk, bv, bo on partition 0
            bdsb = consts.tile([KU, H, KU], f16)
            rbsb = consts.tile([KU, H], f32)
            lrsb = consts.tile([KU, H, 3, 128], f16)
            pkus = consts.tile([128, U], f32)
            mres = consts.tile([128, U], f16)
            pq2s = consts.tile([128, 4], f32)
            pqc2s = consts.tile([128, 4], f32)
            yt = persist.tile([128, 2, N], f16)      # Y^T full batch
            ytq = persist.tile([128, 2, R], f16)     # Y^T my rows

            # DMA issue order: phase-A inputs first so the first projection
            # matmuls can start ASAP, then B0/B1 constants, xp bulk, wos last.
            nc.sync.dma_start(ytq, d_yqt[:, :].rearrange("(t p) n -> p t n", p=128))
            nc.sync.dma_start(yt, d_yt[:, :].rearrange("(t p) n -> p t n", p=128))
            nc.sync.dma_start(wqs, d_wq[:, :].rearrange("(t p) d -> p t d", p=128))
            nc.sync.dma_start(wks, d_wk[:, :].rearrange("(t p) d -> p t d", p=128))
            nc.sync.dma_start(wvs, d_wv[:, :].rearrange("(t p) d -> p t d", p=128))
            nc.sync.dma_start(b4s, d_b4[:, :].rearrange("(p r) d -> p r d", p=1))
            nc.sync.dma_start(pkus, d_pku[:, :])
            nc.sync.dma_start(mres, d_mre[:, :])
            nc.sync.dma_start(pq2s, d_pq2[:, :])
            nc.sync.dma_start(pqc2s, d_pqc2[:, :])
            nc.sync.dma_start(bdsb, d_bd[:, :, :].rearrange("h p m -> p h m"))
            nc.sync.dma_start(rbsb, d_rb[:, :])
            nc.sync.dma_start(lrsb, d_lr[:, :, :, :])

            # ---------- persistent activations ----------
            ktsb = persist.tile([128, 2, 1152], f16)  # K^T, zero-padded keys
            qtz = persist.tile([128, H, 512], f16)    # per-head zero-padded Q^T
            vsb = persist.tile([128, U, D], f16)      # V [key-in-unit, dout]
            v2u = persist.tile([128, U, H, 33], f16)  # [pk*V_h | pk]
            xtall = persist.tile([128, U, 3, 512], f16)  # Xp^T (126 rows used)
            osb = persist.tile([128, 4, D], f32)      # O rows accumulator
            os16 = persist.tile([128, 4, D], f16)     # fp16 copy for O@Wo
            otc = persist.tile([128, 4, 2, 128], f16)  # O^T chunks for O@Wo

            nc.gpsimd.memset(qtz, 0.0)
            nc.gpsimd.memset(ktsb[:, 0, ds(1024, 128)], 0.0)
            nc.gpsimd.memset(ktsb[:, 1, ds(1024, 128)], 0.0)
            nc.gpsimd.memset(xtall[:, 8, 0], 0.0)

            # xp staging: DMA everything up front so it streams during phase A
            xpin = persist.tile([128, U, 4, 384], f16)
            for u in range(8):
                nc.sync.dma_start(
                    xpin[:, u, :, 0:378],
                    d_xp[:, ds(378 * u, 378)].rearrange(
                        "(qt p) c -> p qt c", p=128
                    ),
                )
            nc.sync.dma_start(
                xpin[:, 8, :, 0:48],
                d_xp[:, ds(3024, 48)].rearrange("(qt p) c -> p qt c", p=128),
            )
            nc.sync.dma_start(wos, d_wo[:, :].rearrange("(t p) d -> p t d", p=128))

            # ---------- phase A: projections (+ B0: transpose X_pairs) ----------
            with tc.tile_pool(name="ph_a", bufs=1) as pha, \
                 tc.tile_pool(name="ps_a", bufs=2, space="PSUM") as psa, \
                 tc.tile_pool(name="ps_t", bufs=2, space="PSUM") as pst:
                qtsb = pha.tile([128, 2, R], f16)
                # Q^T (scaled Wq), K^T projections
                for dt_ in range(2):
                    ps = psa.tile([128, 512], f32)
                    for k_ in range(2):
                        nc.tensor.matmul(
                            ps, wqs[:, k_, ds(128 * dt_, 128)], ytq[:, k_],
                            start=(k_ == 0), stop=False,
                        )
                    nc.tensor.matmul(
                        ps, b4s[0:1, 0, ds(128 * dt_, 128)], ones512,
                        start=False, stop=True,
                    )
                    nc.vector.tensor_copy(qtsb[:, dt_], ps)

                    for half in range(2):
                        ps = psa.tile([128, 512], f32)
                        for k_ in range(2):
                            nc.tensor.matmul(
                                ps, wks[:, k_, ds(128 * dt_, 128)],
                                yt[:, k_, ds(512 * half, 512)],
                                start=(k_ == 0), stop=False,
                            )
                        nc.tensor.matmul(
                            ps, b4s[0:1, 1, ds(128 * dt_, 128)], ones512,
                            start=False, stop=True,
                        )
                        nc.vector.tensor_copy(ktsb[:, dt_, ds(512 * half, 512)], ps)

                # V rows per key-unit (feeds V'' and mean V); the bias matmul
                # covers all 128 rows, so pad rows hold plain bv (finite,
                # later killed by pku=0 / mre=0 masks)
                for u in range(U):
                    w = KU if u < 8 else 16
                    ks = ds(KU * u, w)
                    ps = psa.tile([128, 256], f32)
                    for k_ in range(2):
                        nc.tensor.matmul(
                            ps[0:w], yt[:, k_, ks], wvs[:, k_],
                            start=(k_ == 0), stop=False,
                            skip_group_check=True,
                        )
                    nc.tensor.matmul(
                        ps, ones128, b4s[0:1, 2], start=False, stop=True,
                        skip_group_check=True,
                    )
                    nc.vector.tensor_copy(vsb[:, u], ps)

                # V rows for MY queries (residual term), row-major
                vrow = pha.tile([128, 4, D], f32)
                for qc in range(4):
                    ps = psa.tile([128, 256], f32)
                    for k_ in range(2):
                        nc.tensor.matmul(
                            ps, ytq[:, k_, ds(128 * qc, 128)], wvs[:, k_],
                            start=(k_ == 0), stop=False,
                        )
                    nc.tensor.matmul(ps, ones128, b4s[0:1, 2], start=False, stop=True)
                    nc.scalar.copy(vrow[:, qc], ps)

                # per-head zero-padded Q^T slices (keeps content matmuls K=128)
                for h in range(H):
                    base = 32 * (h % 4)
                    nc.gpsimd.tensor_copy(
                        qtz[ds(base, 32), h], qtsb[ds(base, 32), h // 4]
                    )

                # V'' = [pk * V_h | pk] in unit layout
                for u in range(U):
                    nc.gpsimd.tensor_scalar(
                        v2u[:, u, :, 0:32],
                        vsb[:, u].rearrange("p (h d) -> p h d", h=H),
                        pkus[:, u : u + 1],
                        None,
                        op0=OP.mult,
                    )
                    nc.gpsimd.tensor_copy(
                        v2u[:, u, :, 32:33],
                        pkus[:, u : u + 1].to_broadcast((128, H, 1)),
                    )

                # mean_k V as a row (mre = 1/1024 on real keys, 0 on pads),
                # replicated to all partitions; osb init = V_mine + (1-pq)*meanV
                psmr = psa.tile([1, 256], f32)
                for u in range(U):
                    nc.tensor.matmul(
                        psmr, mres[:, u : u + 1], vsb[:, u],
                        start=(u == 0), stop=(u == U - 1),
                    )
                mv16 = pha.tile([1, 256], f16)
                nc.vector.tensor_copy(mv16, psmr)
                psmb = psa.tile([128, 256], f32)
                nc.tensor.matmul(psmb, ones128, mv16, start=True, stop=True)
                for qc in range(4):
                    wall = pha.tile([128, 256], f32)
                    nc.vector.tensor_scalar(
                        wall, psmb, pqc2s[:, qc : qc + 1], None, op0=OP.mult
                    )
                    nc.vector.tensor_add(osb[:, qc], vrow[:, qc], wall)

                # B0: transpose X_pairs into [3kk+c, query] layout per chunk
                for u in range(U):
                    nch = 3 if u < 8 else 1
                    w = KU if u < 8 else 48
                    for c in range(nch):
                        ps = pst.tile([128, 512], f16)
                        for qt in range(4):
                            nc.tensor.transpose(
                                ps[0:w, ds(128 * qt, 128)],
                                xpin[:, u, qt, ds(KU * c, w)],
                                ident,
                            )
                        if (u * 3 + c) % 2 == 0:
                            nc.scalar.copy(xtall[0:w, u, c], ps[0:w])
                        else:
                            nc.vector.tensor_copy(xtall[0:w, u, c], ps[0:w])

            # ---------- phase B1: attention main loop ----------
            with tc.tile_pool(name="ps_ct", bufs=2, space="PSUM") as psct, \
                 tc.tile_pool(name="ps_z", bufs=3, space="PSUM") as psz, \
                 tc.tile_pool(name="ps_av", bufs=2, space="PSUM") as psav, \
                 tc.tile_pool(name="rz_p", bufs=3) as rzp, \
                 tc.tile_pool(name="et_p", bufs=2) as etp, \
                 tc.tile_pool(name="fin_p", bufs=4) as finp:
                for h in range(H):
                    av = psav.tile([128, 4, 33], f32)
                    for u in range(U):
                        nch = 3 if u < 8 else 1
                        ct = psct.tile([128, 512], f32, name="ct", tag="ct")
                        nc.tensor.matmul(
                            ct[0:KU],
                            ktsb[:, h // 4, ds(KU * u, KU)],
                            qtz[:, h],
                            start=True, stop=False,
                        )
                        rzs = []
                        for c in range(nch):
                            zps = psz.tile([128, 512], f32)
                            nc.tensor.matmul(
                                zps[0:KU], bdsb[:, h], xtall[0:KU, u, c],
                                start=True, stop=True,
                            )
                            rz = rzp.tile([128, 512], f16)
                            if (u * 3 + c) % 2 == 0:
                                nc.scalar.activation(
                                    rz[0:KU], zps[0:KU], AF.Relu,
                                    bias=rbsb[:, h : h + 1],
                                )
                            else:
                                nc.vector.tensor_scalar(
                                    rz[0:KU], zps[0:KU], rbsb[:, h : h + 1], 0.0,
                                    op0=OP.add, op1=OP.max,
                                )
                            rzs.append(rz)
                            if c >= 1:
                                # signed reduce of the previous chunk onto ct
                                nc.tensor.matmul(
                                    ct, lrsb[:, h, c - 1], rzs[c - 1][0:KU],
                                    start=False, stop=False,
                                    skip_group_check=True,
                                )
                        nc.tensor.matmul(
                            ct, lrsb[:, h, nch - 1], rzs[nch - 1][0:KU],
                            start=False, stop=True,
                            skip_group_check=True,
                        )
                        et = etp.tile([128, 512], f16)
                        nc.scalar.activation(et[0:KU], ct[0:KU], AF.Exp)
                        # A@V'' with exp chunks stationary -> row-major out.
                        # PSUM `start` resets the WHOLE bank, so only the
                        # first matmul into the bank may carry it.
                        for qc in range(4):
                            nc.tensor.matmul(
                                av[:, qc], et[0:KU, ds(128 * qc, 128)],
                                v2u[0:KU, u, h],
                                start=(u == 0 and qc == 0), stop=(u == U - 1),
                                skip_group_check=True,
                            )
                    # finalize head h: per-query softmax divide + pq blend,
                    # accumulated straight into the row-major O buffer
                    for qc in range(4):
                        rec = finp.tile([128, 1], f32)
                        nc.vector.reciprocal(rec, av[:, qc, 32:33])
                        rcp = finp.tile([128, 1], f32)
                        nc.vector.tensor_mul(rcp, rec, pq2s[:, qc : qc + 1])
                        u_ = finp.tile([128, 32], f32)
                        nc.vector.tensor_scalar(
                            u_, av[:, qc, 0:32], rcp, None, op0=OP.mult
                        )
                        nc.gpsimd.tensor_add(
                            osb[:, qc, ds(32 * h, 32)],
                            osb[:, qc, ds(32 * h, 32)],
                            u_,
                        )

            # ---------- phase C: O = O + relu(O @ Wo + bo) ----------
            with tc.tile_pool(name="ps_o", bufs=2, space="PSUM") as pso, \
                 tc.tile_pool(name="o_p", bufs=2) as op_:
                for qc in range(4):
                    if qc % 2 == 0:
                        nc.scalar.copy(os16[:, qc], osb[:, qc])
                    else:
                        nc.vector.tensor_copy(os16[:, qc], osb[:, qc])
                    pst2 = pso.tile([128, 2, 128], f16)
                    for dt_ in range(2):
                        nc.tensor.transpose(
                            pst2[:, dt_], os16[:, qc, ds(128 * dt_, 128)], ident
                        )
                    if qc % 2 == 0:
                        nc.scalar.copy(otc[:, qc], pst2)
                    else:
                        nc.vector.tensor_copy(otc[:, qc], pst2)
                for qc in range(4):
                    pso2 = pso.tile([128, 256], f32)
                    for dt_ in range(2):
                        nc.tensor.matmul(
                            pso2, otc[:, qc, dt_], wos[:, dt_],
                            start=(dt_ == 0), stop=False,
                        )
                    nc.tensor.matmul(pso2, ones128, b4s[0:1, 3], start=False, stop=True)
                    r2 = op_.tile([128, 256], f32)
                    nc.scalar.activation(r2, pso2, AF.Relu)
                    ofin = op_.tile([128, 256], f32)
                    nc.vector.tensor_add(ofin, osb[:, qc], r2)
                    nc.sync.dma_start(d_o[ds(128 * qc, 128), :], ofin)

    _split_multiwait(nc, mybir)
    return nc


def _split_multiwait(nc, mybir):
    """This walrus build only encodes ONE sem-wait per instruction; Tile's
    tail drain carries several. Split extras onto preceding NoOps."""
    for f in nc.m.functions:
        for blk in f.blocks:
            insts = list(blk.instructions)
            changed = False
            newlist = []
            for ins in insts:
                si = ins.sync_info
                if si is not None and len(si.on_wait) > 1:
                    waits = list(si.on_wait)
                    for j, w in enumerate(waits[:-1]):
                        newlist.append(
                            mybir.InstNoOp(
                                name=f"{ins.name}_splitw{j}",
                                engine=ins.engine,
                                ins=[],
                                outs=[],
                                sync_info=mybir.SyncInfo(on_wait=[w], on_update=[]),
                            )
                        )
                    ins.sync_info = mybir.SyncInfo(
                        on_wait=[waits[-1]], on_update=list(si.on_update)
                    )
                    changed = True
                newlist.append(ins)
            if changed:
                blk.instructions = newlist


def _host_constants(Wg1, bg1, wg2, bg2):
    """Folded block-diag layer-1 weights (fp16), relu biases (fp32) and the
    per-head signed reduce blocks (fp16) in 3-slot / 42-key packing."""
    aw = np.abs(wg2)  # [H, 3]
    sw = np.sign(wg2).astype(np.float32)
    kk = np.arange(42)

    bd = np.zeros((H, KU, KU), np.float16)
    rb = np.zeros((KU, H), np.float32)
    lr = np.zeros((KU, H, 3, 128), np.float16)
    for c in range(3):
        for s in range(3):
            # bd[h, 3kk+c, 3kk+s] = |wg2[h,s]| * Wg1[h,c,s]
            bd[:, 3 * kk + c, 3 * kk + s] = (
                aw[:, s : s + 1] * Wg1[:, c, s : s + 1]
            ).astype(np.float16)
    for s in range(3):
        rb[3 * kk + s, :] = (aw[:, s] * bg1[:, s])[np.newaxis, :]
        for c in range(3):
            # lr[3kk+s, h, c, 42c+kk] = sign(wg2[h, s])
            lr[3 * kk + s, :, c, 42 * c + kk] = sw[:, s][np.newaxis, :].astype(
                np.float16
            )
    return bd, rb, lr


def kernel(**inputs):
    from concourse.bass_utils import run_bass_kernel_spmd

    f16 = np.float16
    Y = np.asarray(inputs["Y_lift"], np.float32)    # [B, N, D]
    XP = np.asarray(inputs["X_pairs"], np.float32)  # [B, N, N, 3]
    PQ = np.asarray(inputs["presence_q"], np.float32)
    PK = np.asarray(inputs["presence_k"], np.float32)

    bd, rb, lr = _host_constants(
        np.asarray(inputs["Wg1"], np.float32),
        np.asarray(inputs["bg1"], np.float32),
        np.asarray(inputs["wg2"], np.float32),
        np.asarray(inputs["bg2"], np.float32),
    )

    XP16 = XP.reshape(B, N, 3 * N).astype(f16)
    YT16 = Y.transpose(0, 2, 1).astype(f16)  # [B, D, N]

    wq = (np.asarray(inputs["Wq"], np.float32) / 16.0).astype(f16)
    wk = np.asarray(inputs["Wk"], np.float32).astype(f16)
    wv = np.asarray(inputs["Wv"], np.float32).astype(f16)
    wo = np.asarray(inputs["Wo"], np.float32).astype(f16)
    b4 = np.stack(
        [
            np.asarray(inputs["bq"], np.float32) / 16.0,
            np.asarray(inputs["bk"], np.float32),
            np.asarray(inputs["bv"], np.float32),
            np.asarray(inputs["bo"], np.float32),
        ]
    ).astype(f16)

    # 1/1024 mean-weights over real keys, in 126-key unit layout
    mre = np.zeros((128, U), f16)
    mre[:KU] = (
        (np.arange(U * KU).reshape(U, KU).T < N).astype(np.float32) / 1024.0
    ).astype(f16)

    if "nc" not in _CACHE:
        _CACHE["nc"] = _build_program()
    nc = _CACHE["nc"]

    in_maps = []
    for core in range(NCORES):
        b, half = core // 2, core % 2
        rows = slice(half * R, half * R + R)
        pq2 = np.ascontiguousarray(PQ[b, rows].reshape(4, 128).T)
        pkpad = np.zeros(U * KU, np.float32)
        pkpad[:N] = PK[b]
        pku = np.zeros((128, U), np.float32)
        pku[:KU] = pkpad.reshape(U, KU).T
        in_maps.append(
            {
                "yt": YT16[b],
                "yqt": YT16[b][:, rows],
                "xp": XP16[b, rows],
                "pku": pku,
                "mre": mre,
                "pq2": pq2,
                "pqc2": 1.0 - pq2,
                "wq": wq,
                "wk": wk,
                "wv": wv,
                "wo": wo,
                "b4": b4,
                "bd": bd,
                "rb": rb,
                "lr": lr,
            }
        )

    res = run_bass_kernel_spmd(nc, in_maps, core_ids=list(range(NCORES)))
    out = np.empty((B, N, D), np.float32)
    for core in range(NCORES):
        b, half = core // 2, core % 2
        out[b, half * R : half * R + R] = res.results[core]["o"]
    return out
